# revision 6
# baseline (speedup 1.0000x reference)
"""Trainium2 Bass kernel for the 2-layer GRU decoder with categorical sampling.

Strategy (8 NeuronCores, data-parallel over batch):
- Shard the batch (256) into 8 x 32 rows; replicate all weights. The scan's
  sequential dependency stays on-core; zero collectives.
- The categorical sampling uses jax.random.key(42) -> split(T) -> gumbel,
  which is input-independent; the Gumbel noise G [T, B, V] is precomputed on
  host (CPU) and the kernel samples via onehot(argmax(logits + g_t)) using a
  DVE max + is_equal compare, feeding the next embedding through a tiny
  PE matmul (emb^T @ onehot^T).
- All matmuls run in fp32 (weights-moving: stationary activation chunks
  [128, 32], weight rows streamed as [128, 512] rhs slices, fp32 PSUM
  accumulation) so the logits stay at the fp32 noise floor (~1e-6): the
  token trajectory exactly matches the reference (argmax gaps down to 2e-6
  would flip under any lower-precision scheme).
- Layer-0 weights + w_out/emb stay SBUF-resident; layer-1 weights (25 MB,
  over SBUF capacity) are re-streamed from HBM each step, overlapped with
  compute.
- Tokens are reconstructed on host as argmax(logits + G): bit-identical to
  the kernel's internal onehot path.
"""
import sys

sys.path.insert(0, "/opt/trn_rl_repo")
import numpy as np

import concourse.bass as bass
import concourse.mybir as mybir
from concourse.tile import TileContext
from concourse.bass_utils import run_bass_kernel_spmd
from concourse.vector_clock import ScopedClock
import concourse.tile as tile_mod

F32 = mybir.dt.float32
AF = mybir.ActivationFunctionType
ALU = mybir.AluOpType

B, H, E, V, Z, T = 32, 1024, 256, 28, 1024, 512  # B = per-core batch
BF = 256  # full batch
KH, KE = 8, 2
GW = 512
SOS = 1

# ---------------------------------------------------------------------------
# Workaround: walrus in this toolchain accepts only ONE sync-wait per TPB
# instruction. Split extras onto preceding same-engine nop carriers, and chunk
# the TileContext final-drain waits.
# ---------------------------------------------------------------------------
_TPB = None


def _tpb():
    global _TPB
    if _TPB is None:
        ET = mybir.EngineType
        _TPB = {ET.PE, ET.Activation, ET.Pool, ET.DVE, ET.SP}
    return _TPB


def _patched_drain_and_barrier(self, tick_clock, wait_clock):
    nc = self.nc
    drain_inst = nc.sync.drain()
    wait_clock.add_sem_waits(drain_inst.ins, ScopedClock({None: tick_clock.global_clock}))
    si = drain_inst.ins.sync_info
    waits = list(si.on_wait) if si is not None else []
    if len(waits) > 1:
        drain_inst.ins.sync_info = mybir.SyncInfo(on_wait=waits[:1], on_update=list(si.on_update))
        for i in range(1, len(waits)):
            carrier = nc.sync.drain()
            carrier.ins.sync_info = mybir.SyncInfo(on_wait=waits[i : i + 1], on_update=[])
    nc.all_engine_barrier()
    assert self.sems is not None
    popped = nc._tile_sem_poison_stack.pop()
    assert popped is self._sem_poison
    nc.clear_and_free_semaphores(list(self.sems.allocated().values()))
    nc.all_engine_barrier()


_orig_commit = tile_mod.TileContext._commit_instruction


def _patched_commit(self, inst, lazy_reg_writes=True):
    si = getattr(inst, "sync_info", None)
    eng = getattr(inst, "engine", None)
    if si is not None and si.on_wait and len(si.on_wait) > 1 and eng in _tpb():
        waits = list(si.on_wait)
        for w in waits[:-1]:
            carrier = mybir.InstNoOp(
                name=self.nc.get_next_instruction_name(),
                engine=eng,
                sync_info=mybir.SyncInfo(on_wait=[w], on_update=[]),
                bass_nofuse=True,
            )
            _orig_commit(self, carrier, lazy_reg_writes)
        inst.sync_info = mybir.SyncInfo(on_wait=[waits[-1]], on_update=list(si.on_update))
    return _orig_commit(self, inst, lazy_reg_writes)


def _apply_patches():
    tile_mod.TileContext._drain_and_barrier = _patched_drain_and_barrier
    tile_mod.TileContext._commit_instruction = _patched_commit


# ---------------------------------------------------------------------------
# Bass program
# ---------------------------------------------------------------------------

def _transpose_to_T(nc, src, dst, width):
    """src [32, width] -> dst [128, (width//128)*32], dst[p, k*32+b] = src[b, k*128+p]."""
    for j in range(4):
        in_ap = src.rearrange("b (k jc) -> b k jc", jc=128)[:, :, j * 32 : (j + 1) * 32]
        out_ap = dst[32 * j : 32 * (j + 1), :].rearrange("c (k b) -> c k b", b=32)
        nc.vector.transpose(out_ap, in_ap)


def build_decoder(T_=T, INNER=64, with_bias=False):
    _apply_patches()
    assert T_ % INNER == 0
    n_blocks = T_ // INNER
    nc = bass.Bass()
    P = lambda name, shape, out=False: nc.declare_dram_parameter(name, shape, F32, isOutput=out)

    z_in = P("z", [B, Z])
    g_in = P("g", [B, T_ * V])
    wzhT = P("wzhT", [128, 8 * 2048])
    wihT0 = P("wihT0", [128, KE * 3 * H])
    whhT0 = P("whhT0", [128, KH * 3 * H])
    wihT1 = P("wihT1", [128, KH * 3 * H])
    whhT1 = P("whhT1", [128, KH * 3 * H])
    woutT = P("woutT", [128, KH * V])
    embL = P("embL", [V, E])
    x0T = P("x0T", [128, KE * B])
    lout = P("lout", [B, T_ * V], out=True)
    if with_bias:
        b_rz0 = P("b_rz0", [1, 2048])
        b_in0 = P("b_in0", [1, H])
        b_hn0 = P("b_hn0", [1, H])
        b_rz1 = P("b_rz1", [1, 2048])
        b_in1 = P("b_in1", [1, H])
        b_hn1 = P("b_hn1", [1, H])
        b_zh = P("b_zh", [1, 2048])
        b_out = P("b_out", [1, V])

    with TileContext(nc) as tc:
        with (
            tc.tile_pool(name="resw", bufs=1) as resw,
            tc.tile_pool(name="state", bufs=1) as state,
            tc.tile_pool(name="scr", bufs=2) as scr,
            tc.tile_pool(name="scrs", bufs=2) as scrs,
            tc.tile_pool(name="ws", bufs=3) as wsp,
            tc.tile_pool(name="gl", bufs=1) as glp,
            tc.tile_pool(name="ps", bufs=6, space="PSUM") as psp,
            tc.tile_pool(name="pss", bufs=1, space="PSUM") as pssp,
        ):
            whhT0_sb = resw.tile([128, KH * 3 * H], F32, tag="whhT0")
            wihT0_sb = resw.tile([128, KE * 3 * H], F32, tag="wihT0")
            woutT_sb = resw.tile([128, KH * V], F32, tag="woutT")
            embL_sb = resw.tile([V, E], F32, tag="embL")
            nc.sync.dma_start(out=whhT0_sb[:], in_=whhT0[:])
            nc.sync.dma_start(out=wihT0_sb[:], in_=wihT0[:])
            nc.sync.dma_start(out=woutT_sb[:], in_=woutT[:])
            nc.sync.dma_start(out=embL_sb[:], in_=embL[:])
            if with_bias:
                ones_sb = resw.tile([1, B], F32, tag="ones")
                nc.vector.memset(ones_sb[:], 1.0)
                bias_sb = {}
                for nm, src, wdt in (
                    ("rz0", b_rz0, 2048), ("in0", b_in0, H), ("hn0", b_hn0, H),
                    ("rz1", b_rz1, 2048), ("in1", b_in1, H), ("hn1", b_hn1, H),
                    ("zh", b_zh, 2048), ("out", b_out, V),
                ):
                    t = resw.tile([1, wdt], F32, tag=f"b_{nm}")
                    nc.sync.dma_start(out=t[:], in_=src[:])
                    bias_sb[nm] = t

            h0 = state.tile([B, H], F32, tag="h0")
            h1 = state.tile([B, H], F32, tag="h1")
            h0T = state.tile([128, KH * B], F32, tag="h0T")
            h1T = state.tile([128, KH * B], F32, tag="h1T")
            xT = state.tile([128, KE * B], F32, tag="xT")
            onehot = state.tile([B, 32], F32, tag="onehot")
            nc.vector.memset(onehot[:], 0.0)
            nc.sync.dma_start(out=xT[:], in_=x0T[:])

            # init h0/h1 = tanh(z @ w_zh.T + b_zh)
            z_sb = wsp.tile([B, Z], F32, tag="wstr")
            nc.sync.dma_start(out=z_sb[:], in_=z_in[:])
            zT = wsp.tile([128, 8 * B], F32, tag="wstr")
            _transpose_to_T(nc, z_sb[:], zT, Z)
            for c in range(4):
                ps = psp.tile([B, GW], F32, tag="ps")
                first = True
                if with_bias:
                    nc.tensor.matmul(ps[:], ones_sb[:], bias_sb["zh"][:, c * GW : (c + 1) * GW], start=True, stop=False)
                    first = False
                for k in range(8):
                    wzk = wsp.tile([128, GW], F32, tag="wstr")
                    nc.sync.dma_start(out=wzk[:], in_=wzhT[:, k * 2048 + c * GW : k * 2048 + (c + 1) * GW])
                    nc.tensor.matmul(ps[:], zT[:, k * B : (k + 1) * B], wzk[:], start=first, stop=(k == 7))
                    first = False
                dst = h0 if c < 2 else h1
                nc.scalar.activation(dst[:, (c % 2) * GW : (c % 2) * GW + GW], ps[:], AF.Tanh)
            _transpose_to_T(nc, h0[:], h0T, H)
            _transpose_to_T(nc, h1[:], h1T, H)

            def wih0_slices(c):
                return [wihT0_sb[:, k * 3 * H + c * GW : k * 3 * H + (c + 1) * GW] for k in range(KE)]

            def whh0_slices(c):
                return [whhT0_sb[:, k * 3 * H + c * GW : k * 3 * H + (c + 1) * GW] for k in range(KH)]

            def gru_layer0():
                rz = scr.tile([B, 2048], F32, tag="rz", name="rz0")
                for c in range(4):
                    ps = psp.tile([B, GW], F32, tag="ps", name="ps_rz0")
                    first = True
                    if with_bias:
                        nc.tensor.matmul(ps[:], ones_sb[:], bias_sb["rz0"][:, c * GW : c * GW + GW], start=True, stop=False)
                        first = False
                    whs = whh0_slices(c)
                    for k in range(KH):
                        nc.tensor.matmul(ps[:], h0T[:, k * B : (k + 1) * B], whs[k], start=first, stop=False)
                        first = False
                    wis = wih0_slices(c)
                    for k in range(KE):
                        nc.tensor.matmul(ps[:], xT[:, k * B : (k + 1) * B], wis[k], start=False, stop=(k == KE - 1))
                    nc.scalar.activation(rz[:, c * GW : (c + 1) * GW], ps[:], AF.Sigmoid)
                for c in range(2):
                    cg = c + 4
                    ps_i = psp.tile([B, GW], F32, tag="ps", name="ps_i0")
                    first = True
                    if with_bias:
                        nc.tensor.matmul(ps_i[:], ones_sb[:], bias_sb["in0"][:, c * GW : c * GW + GW], start=True, stop=False)
                        first = False
                    wis = wih0_slices(cg)
                    for k in range(KE):
                        nc.tensor.matmul(ps_i[:], xT[:, k * B : (k + 1) * B], wis[k], start=first, stop=(k == KE - 1))
                        first = False
                    ps_h = psp.tile([B, GW], F32, tag="ps", name="ps_h0")
                    first = True
                    if with_bias:
                        nc.tensor.matmul(ps_h[:], ones_sb[:], bias_sb["hn0"][:, c * GW : c * GW + GW], start=True, stop=False)
                        first = False
                    whs = whh0_slices(cg)
                    for k in range(KH):
                        nc.tensor.matmul(ps_h[:], h0T[:, k * B : (k + 1) * B], whs[k], start=first, stop=(k == KH - 1))
                        first = False
                    tmp = scrs.tile([B, GW], F32, tag="tmp", name="tmp0")
                    nc.vector.tensor_tensor(tmp[:], rz[:, c * GW : (c + 1) * GW], ps_h[:], ALU.mult)
                    nc.vector.tensor_tensor(tmp[:], tmp[:], ps_i[:], ALU.add)
                    n_sb = scrs.tile([B, GW], F32, tag="n_sb", name="n0")
                    nc.scalar.activation(n_sb[:], tmp[:], AF.Tanh)
                    d = scrs.tile([B, GW], F32, tag="d", name="d0")
                    hs = h0[:, c * GW : (c + 1) * GW]
                    nc.vector.tensor_tensor(d[:], hs, n_sb[:], ALU.subtract)
                    nc.vector.tensor_tensor(d[:], rz[:, 1024 + c * GW : 1024 + (c + 1) * GW], d[:], ALU.mult)
                    nc.vector.tensor_tensor(hs, n_sb[:], d[:], ALU.add)
                _transpose_to_T(nc, h0[:], h0T, H)

            for blk in range(n_blocks):
                g_blk = glp.tile([B, INNER * V], F32, tag="g_blk")
                nc.sync.dma_start(out=g_blk[:], in_=g_in[:, blk * INNER * V : (blk + 1) * INNER * V])
                l_blk = glp.tile([B, INNER * V], F32, tag="l_blk")

                with tc.For_i(0, INNER * V, V) as iv:
                    gru_layer0()

                    # layer 1: k-outer streaming from HBM
                    rz1 = scr.tile([B, 2048], F32, tag="rz", name="rz1")
                    psA = [psp.tile([B, GW], F32, tag="ps", name=f"psA{i}") for i in range(4)]
                    if with_bias:
                        for c in range(4):
                            nc.tensor.matmul(psA[c][:], ones_sb[:], bias_sb["rz1"][:, c * GW : (c + 1) * GW], start=True, stop=False)
                    for k in range(KH):
                        whk = wsp.tile([128, 2048], F32, tag="wstr", name="whk")
                        nc.sync.dma_start(out=whk[:], in_=whhT1[:, k * 3 * H : k * 3 * H + 2048])
                        for c in range(4):
                            nc.tensor.matmul(psA[c][:], h1T[:, k * B : (k + 1) * B], whk[:, c * GW : (c + 1) * GW], start=(k == 0 and not with_bias), stop=False)
                    for k in range(KH):
                        wik = wsp.tile([128, 2048], F32, tag="wstr", name="wik")
                        nc.sync.dma_start(out=wik[:], in_=wihT1[:, k * 3 * H : k * 3 * H + 2048])
                        for c in range(4):
                            nc.tensor.matmul(psA[c][:], h0T[:, k * B : (k + 1) * B], wik[:, c * GW : (c + 1) * GW], start=False, stop=(k == KH - 1))
                    for c in range(4):
                        nc.scalar.activation(rz1[:, c * GW : (c + 1) * GW], psA[c][:], AF.Sigmoid)

                    psI = [psp.tile([B, GW], F32, tag="ps", name=f"psI{i}") for i in range(2)]
                    psH = [psp.tile([B, GW], F32, tag="ps", name=f"psH{i}") for i in range(2)]
                    if with_bias:
                        for c in range(2):
                            nc.tensor.matmul(psI[c][:], ones_sb[:], bias_sb["in1"][:, c * GW : (c + 1) * GW], start=True, stop=False)
                            nc.tensor.matmul(psH[c][:], ones_sb[:], bias_sb["hn1"][:, c * GW : (c + 1) * GW], start=True, stop=False)
                    for k in range(KH):
                        whkn = wsp.tile([128, 1024], F32, tag="wstr", name="whkn")
                        nc.sync.dma_start(out=whkn[:], in_=whhT1[:, k * 3 * H + 2048 : (k + 1) * 3 * H])
                        for c in range(2):
                            nc.tensor.matmul(psH[c][:], h1T[:, k * B : (k + 1) * B], whkn[:, c * GW : (c + 1) * GW], start=(k == 0 and not with_bias), stop=(k == KH - 1))
                    for k in range(KH):
                        wikn = wsp.tile([128, 1024], F32, tag="wstr", name="wikn")
                        nc.sync.dma_start(out=wikn[:], in_=wihT1[:, k * 3 * H + 2048 : (k + 1) * 3 * H])
                        for c in range(2):
                            nc.tensor.matmul(psI[c][:], h0T[:, k * B : (k + 1) * B], wikn[:, c * GW : (c + 1) * GW], start=(k == 0 and not with_bias), stop=(k == KH - 1))
                    for c in range(2):
                        tmp = scrs.tile([B, GW], F32, tag="tmp", name="tmp1")
                        nc.vector.tensor_tensor(tmp[:], rz1[:, c * GW : (c + 1) * GW], psH[c][:], ALU.mult)
                        nc.vector.tensor_tensor(tmp[:], tmp[:], psI[c][:], ALU.add)
                        n_sb = scrs.tile([B, GW], F32, tag="n_sb", name="n1")
                        nc.scalar.activation(n_sb[:], tmp[:], AF.Tanh)
                        d = scrs.tile([B, GW], F32, tag="d", name="d1")
                        hs = h1[:, c * GW : (c + 1) * GW]
                        nc.vector.tensor_tensor(d[:], hs, n_sb[:], ALU.subtract)
                        nc.vector.tensor_tensor(d[:], rz1[:, 1024 + c * GW : 1024 + (c + 1) * GW], d[:], ALU.mult)
                        nc.vector.tensor_tensor(hs, n_sb[:], d[:], ALU.add)
                    _transpose_to_T(nc, h1[:], h1T, H)

                    # logits
                    ps_l = pssp.tile([B, V], F32, tag="ps_l")
                    first = True
                    if with_bias:
                        nc.tensor.matmul(ps_l[:], ones_sb[:], bias_sb["out"][:], start=True, stop=False)
                        first = False
                    for k in range(KH):
                        nc.tensor.matmul(ps_l[:], h1T[:, k * B : (k + 1) * B], woutT_sb[:, k * V : (k + 1) * V], start=first, stop=(k == KH - 1))
                        first = False
                    nc.scalar.activation(l_blk[:, bass.ds(iv, V)], ps_l[:], AF.Copy)

                    # sample next token as onehot(argmax(logits + g))
                    s_sb = scrs.tile([B, V], F32, tag="s_sb")
                    nc.vector.tensor_tensor(s_sb[:], ps_l[:], g_blk[:, bass.ds(iv, V)], ALU.add)
                    m_sb = scrs.tile([B, 8], F32, tag="m_sb")
                    nc.vector.max(m_sb[:], s_sb[:])
                    nc.vector.tensor_scalar(onehot[:, 0:V], s_sb[:], m_sb[:, 0:1], None, ALU.is_equal)
                    ohT = scrs.tile([32, 32], F32, tag="ohT")
                    nc.vector.transpose(ohT[:], onehot[:])

                    # x = emb[tok] = emb^T @ onehot^T
                    for m in range(KE):
                        ps_x = pssp.tile([128, B], F32, tag="ps_x")
                        nc.tensor.matmul(ps_x[:], embL_sb[:, m * 128 : (m + 1) * 128], ohT[0:V, :], start=True, stop=True)
                        nc.scalar.activation(xT[:, m * B : (m + 1) * B], ps_x[:], AF.Copy)

                nc.sync.dma_start(out=lout[:, blk * INNER * V : (blk + 1) * INNER * V], in_=l_blk[:])

    return nc


# ---------------------------------------------------------------------------
# Host-side prep
# ---------------------------------------------------------------------------

def _chunkT(w, kchunks):
    R_, K = w.shape
    assert K == kchunks * 128
    out = np.empty((128, kchunks * R_), np.float32)
    for k in range(kchunks):
        out[:, k * R_ : (k + 1) * R_] = w[:, k * 128 : (k + 1) * 128].T
    return out


def _gumbel_noise():
    """G [T, 256, V]: the exact noise jax.random.categorical(keys[t], .) uses."""
    import jax
    import jax.numpy as jnp

    cpu = jax.devices("cpu")[0]
    with jax.default_device(cpu):
        keys = jax.random.split(jax.random.key(42), T)
        gfn = jax.jit(lambda k: jax.random.gumbel(k, (BF, V), jnp.float32), backend="cpu")
        return np.stack([np.asarray(gfn(keys[t])) for t in range(T)])


def _host_inputs(inputs, G, core, shared):
    sl = slice(core * B, (core + 1) * B)
    g = np.ascontiguousarray(G[:, sl, :].transpose(1, 0, 2)).reshape(B, T * V)
    m = {"z": np.ascontiguousarray(np.asarray(inputs["z"], np.float32)[sl]), "g": g}
    m.update(shared)
    return m


def _shared_inputs(inputs):
    emb = np.asarray(inputs["emb"], np.float32)
    x0 = emb[SOS]
    x0T = np.empty((128, KE * B), np.float32)
    for k in range(KE):
        x0T[:, k * B : (k + 1) * B] = np.repeat(x0[k * 128 : (k + 1) * 128][:, None], B, axis=1)
    return {
        "wzhT": _chunkT(np.asarray(inputs["w_zh"], np.float32), 8),
        "wihT0": _chunkT(np.asarray(inputs["w_ih0"], np.float32), KE),
        "whhT0": _chunkT(np.asarray(inputs["w_hh0"], np.float32), KH),
        "wihT1": _chunkT(np.asarray(inputs["w_ih1"], np.float32), KH),
        "whhT1": _chunkT(np.asarray(inputs["w_hh1"], np.float32), KH),
        "woutT": _chunkT(np.asarray(inputs["w_out"], np.float32), KH),
        "embL": np.ascontiguousarray(emb),
        "x0T": x0T,
    }


def _bias_inputs(inputs):
    out = {}
    for nm, ki, kh in (("0", "b_ih0", "b_hh0"), ("1", "b_ih1", "b_hh1")):
        bi = np.asarray(inputs[ki], np.float32)
        bh = np.asarray(inputs[kh], np.float32)
        out[f"b_rz{nm}"] = np.ascontiguousarray((bi[: 2 * H] + bh[: 2 * H])[None, :])
        out[f"b_in{nm}"] = np.ascontiguousarray(bi[2 * H :][None, :])
        out[f"b_hn{nm}"] = np.ascontiguousarray(bh[2 * H :][None, :])
    out["b_zh"] = np.ascontiguousarray(np.asarray(inputs["b_zh"], np.float32)[None, :])
    out["b_out"] = np.ascontiguousarray(np.asarray(inputs["b_out"], np.float32)[None, :])
    return out


_CACHE = {}


def kernel(**inputs):
    biases = [inputs[k] for k in ("b_zh", "b_ih0", "b_hh0", "b_ih1", "b_hh1", "b_out")]
    with_bias = any(np.any(np.asarray(b)) for b in biases)

    key = ("nc", with_bias)
    if key not in _CACHE:
        _CACHE[key] = build_decoder(T_=T, INNER=64, with_bias=with_bias)
    nc = _CACHE[key]

    if "G" not in _CACHE:
        _CACHE["G"] = _gumbel_noise()
    G = _CACHE["G"]

    shared = _shared_inputs(inputs)
    if with_bias:
        shared.update(_bias_inputs(inputs))
    in_maps = [_host_inputs(inputs, G, c, shared) for c in range(8)]
    res = run_bass_kernel_spmd(nc, in_maps, list(range(8)))

    logits = np.zeros((BF, T, V), np.float32)
    for c in range(8):
        logits[c * B : (c + 1) * B] = res.results[c]["lout"].reshape(B, T, V)
    tokens = np.argmax(logits.transpose(1, 0, 2) + G, axis=-1).T.astype(np.int32)
    return logits, tokens


# revision 7
# speedup vs baseline: 1.0587x; 1.0587x over previous
"""Trainium2 Bass kernel for the 2-layer GRU decoder with categorical sampling.

Strategy (8 NeuronCores, data-parallel over batch):
- Shard the batch (256) into 8 x 32 rows; replicate all weights. The scan's
  sequential dependency stays on-core; zero collectives.
- The categorical sampling uses jax.random.key(42) -> split(T) -> gumbel,
  which is input-independent; the Gumbel noise G [T, B, V] is precomputed on
  host (CPU) and the kernel samples via onehot(argmax(logits + g_t)) using a
  DVE max + is_equal compare, feeding the next embedding through a tiny
  PE matmul (emb^T @ onehot^T).
- All matmuls run in fp32 (weights-moving: stationary activation chunks
  [128, 32], weight rows streamed as [128, 512] rhs slices, fp32 PSUM
  accumulation) so the logits stay at the fp32 noise floor (~1e-6): the
  token trajectory exactly matches the reference (argmax gaps down to 2e-6
  would flip under any lower-precision scheme).
- Layer-0 weights + w_out/emb stay SBUF-resident; layer-1 weights (25 MB,
  over SBUF capacity) are re-streamed from HBM each step, overlapped with
  compute.
- Tokens are reconstructed on host as argmax(logits + G): bit-identical to
  the kernel's internal onehot path.
"""
import sys

sys.path.insert(0, "/opt/trn_rl_repo")
import numpy as np

import concourse.bass as bass
import concourse.mybir as mybir
from concourse.tile import TileContext
from concourse.bass_utils import run_bass_kernel_spmd
from concourse.vector_clock import ScopedClock
import concourse.tile as tile_mod

F32 = mybir.dt.float32
AF = mybir.ActivationFunctionType
ALU = mybir.AluOpType

B, H, E, V, Z, T = 32, 1024, 256, 28, 1024, 512  # B = per-core batch
BF = 256  # full batch
KH, KE = 8, 2
GW = 512
SOS = 1

# ---------------------------------------------------------------------------
# Workaround: walrus in this toolchain accepts only ONE sync-wait per TPB
# instruction. Split extras onto preceding same-engine nop carriers, and chunk
# the TileContext final-drain waits.
# ---------------------------------------------------------------------------
_TPB = None


def _tpb():
    global _TPB
    if _TPB is None:
        ET = mybir.EngineType
        _TPB = {ET.PE, ET.Activation, ET.Pool, ET.DVE, ET.SP}
    return _TPB


def _patched_drain_and_barrier(self, tick_clock, wait_clock):
    nc = self.nc
    drain_inst = nc.sync.drain()
    wait_clock.add_sem_waits(drain_inst.ins, ScopedClock({None: tick_clock.global_clock}))
    si = drain_inst.ins.sync_info
    waits = list(si.on_wait) if si is not None else []
    if len(waits) > 1:
        drain_inst.ins.sync_info = mybir.SyncInfo(on_wait=waits[:1], on_update=list(si.on_update))
        for i in range(1, len(waits)):
            carrier = nc.sync.drain()
            carrier.ins.sync_info = mybir.SyncInfo(on_wait=waits[i : i + 1], on_update=[])
    nc.all_engine_barrier()
    assert self.sems is not None
    popped = nc._tile_sem_poison_stack.pop()
    assert popped is self._sem_poison
    nc.clear_and_free_semaphores(list(self.sems.allocated().values()))
    nc.all_engine_barrier()


_orig_commit = tile_mod.TileContext._commit_instruction


def _patched_commit(self, inst, lazy_reg_writes=True):
    si = getattr(inst, "sync_info", None)
    eng = getattr(inst, "engine", None)
    if si is not None and si.on_wait and len(si.on_wait) > 1 and eng in _tpb():
        waits = list(si.on_wait)
        for w in waits[:-1]:
            carrier = mybir.InstNoOp(
                name=self.nc.get_next_instruction_name(),
                engine=eng,
                sync_info=mybir.SyncInfo(on_wait=[w], on_update=[]),
                bass_nofuse=True,
            )
            _orig_commit(self, carrier, lazy_reg_writes)
        inst.sync_info = mybir.SyncInfo(on_wait=[waits[-1]], on_update=list(si.on_update))
    return _orig_commit(self, inst, lazy_reg_writes)


def _apply_patches():
    tile_mod.TileContext._drain_and_barrier = _patched_drain_and_barrier
    tile_mod.TileContext._commit_instruction = _patched_commit


# ---------------------------------------------------------------------------
# Bass program
# ---------------------------------------------------------------------------

def _transpose_to_T(nc, src, dst, width):
    """src [32, width] -> dst [128, (width//128)*32], dst[p, k*32+b] = src[b, k*128+p]."""
    for j in range(4):
        in_ap = src.rearrange("b (k jc) -> b k jc", jc=128)[:, :, j * 32 : (j + 1) * 32]
        out_ap = dst[32 * j : 32 * (j + 1), :].rearrange("c (k b) -> c k b", b=32)
        nc.vector.transpose(out_ap, in_ap)


def build_decoder(T_=T, INNER=64, with_bias=False):
    _apply_patches()
    assert T_ % INNER == 0
    n_blocks = T_ // INNER
    nc = bass.Bass()
    P = lambda name, shape, out=False: nc.declare_dram_parameter(name, shape, F32, isOutput=out)

    z_in = P("z", [B, Z])
    g_in = P("g", [B, T_ * V])
    wzhT = P("wzhT", [128, 8 * 2048])
    wihT0 = P("wihT0", [128, KE * 3 * H])
    whhT0 = P("whhT0", [128, KH * 3 * H])
    wihT1 = P("wihT1", [128, KH * 3 * H])
    whhT1 = P("whhT1", [128, KH * 3 * H])
    woutT = P("woutT", [128, KH * V])
    embL = P("embL", [V, E])
    x0T = P("x0T", [128, KE * B])
    lout = P("lout", [B, T_ * V], out=True)
    if with_bias:
        b_rz0 = P("b_rz0", [1, 2048])
        b_in0 = P("b_in0", [1, H])
        b_hn0 = P("b_hn0", [1, H])
        b_rz1 = P("b_rz1", [1, 2048])
        b_in1 = P("b_in1", [1, H])
        b_hn1 = P("b_hn1", [1, H])
        b_zh = P("b_zh", [1, 2048])
        b_out = P("b_out", [1, V])

    with TileContext(nc) as tc:
        with (
            tc.tile_pool(name="resw", bufs=1) as resw,
            tc.tile_pool(name="state", bufs=1) as state,
            tc.tile_pool(name="scr", bufs=2) as scr,
            tc.tile_pool(name="scrs", bufs=2) as scrs,
            tc.tile_pool(name="ws", bufs=6) as wsp,
            tc.tile_pool(name="gl", bufs=1) as glp,
            tc.tile_pool(name="ps", bufs=6, space="PSUM") as psp,
            tc.tile_pool(name="pss", bufs=1, space="PSUM") as pssp,
        ):
            whhT0_sb = resw.tile([128, KH * 3 * H], F32, tag="whhT0")
            wihT0_sb = resw.tile([128, KE * 3 * H], F32, tag="wihT0")
            woutT_sb = resw.tile([128, KH * V], F32, tag="woutT")
            embL_sb = resw.tile([V, E], F32, tag="embL")
            nc.sync.dma_start(out=whhT0_sb[:], in_=whhT0[:])
            nc.sync.dma_start(out=wihT0_sb[:], in_=wihT0[:])
            nc.sync.dma_start(out=woutT_sb[:], in_=woutT[:])
            nc.sync.dma_start(out=embL_sb[:], in_=embL[:])
            if with_bias:
                ones_sb = resw.tile([1, B], F32, tag="ones")
                nc.vector.memset(ones_sb[:], 1.0)
                bias_sb = {}
                for nm, src, wdt in (
                    ("rz0", b_rz0, 2048), ("in0", b_in0, H), ("hn0", b_hn0, H),
                    ("rz1", b_rz1, 2048), ("in1", b_in1, H), ("hn1", b_hn1, H),
                    ("zh", b_zh, 2048), ("out", b_out, V),
                ):
                    t = resw.tile([1, wdt], F32, tag=f"b_{nm}")
                    nc.sync.dma_start(out=t[:], in_=src[:])
                    bias_sb[nm] = t

            h0 = state.tile([B, H], F32, tag="h0")
            h1 = state.tile([B, H], F32, tag="h1")
            h0T = state.tile([128, KH * B], F32, tag="h0T")
            h1T = state.tile([128, KH * B], F32, tag="h1T")
            xT = state.tile([128, KE * B], F32, tag="xT")
            onehot = state.tile([B, 32], F32, tag="onehot")
            nc.vector.memset(onehot[:], 0.0)
            nc.sync.dma_start(out=xT[:], in_=x0T[:])

            # init h0/h1 = tanh(z @ w_zh.T + b_zh)
            z_sb = wsp.tile([B, Z], F32, tag="wstr")
            nc.sync.dma_start(out=z_sb[:], in_=z_in[:])
            zT = wsp.tile([128, 8 * B], F32, tag="wstr")
            _transpose_to_T(nc, z_sb[:], zT, Z)
            for c in range(4):
                ps = psp.tile([B, GW], F32, tag="ps")
                first = True
                if with_bias:
                    nc.tensor.matmul(ps[:], ones_sb[:], bias_sb["zh"][:, c * GW : (c + 1) * GW], start=True, stop=False)
                    first = False
                for k in range(8):
                    wzk = wsp.tile([128, GW], F32, tag="wstr")
                    nc.sync.dma_start(out=wzk[:], in_=wzhT[:, k * 2048 + c * GW : k * 2048 + (c + 1) * GW])
                    nc.tensor.matmul(ps[:], zT[:, k * B : (k + 1) * B], wzk[:], start=first, stop=(k == 7))
                    first = False
                dst = h0 if c < 2 else h1
                nc.scalar.activation(dst[:, (c % 2) * GW : (c % 2) * GW + GW], ps[:], AF.Tanh)
            _transpose_to_T(nc, h0[:], h0T, H)
            _transpose_to_T(nc, h1[:], h1T, H)

            def wih0_slices(c):
                return [wihT0_sb[:, k * 3 * H + c * GW : k * 3 * H + (c + 1) * GW] for k in range(KE)]

            def whh0_slices(c):
                return [whhT0_sb[:, k * 3 * H + c * GW : k * 3 * H + (c + 1) * GW] for k in range(KH)]

            def gru_layer0():
                rz = scr.tile([B, 2048], F32, tag="rz", name="rz0")
                for c in range(4):
                    ps = psp.tile([B, GW], F32, tag="ps", name="ps_rz0")
                    first = True
                    if with_bias:
                        nc.tensor.matmul(ps[:], ones_sb[:], bias_sb["rz0"][:, c * GW : c * GW + GW], start=True, stop=False)
                        first = False
                    whs = whh0_slices(c)
                    for k in range(KH):
                        nc.tensor.matmul(ps[:], h0T[:, k * B : (k + 1) * B], whs[k], start=first, stop=False)
                        first = False
                    wis = wih0_slices(c)
                    for k in range(KE):
                        nc.tensor.matmul(ps[:], xT[:, k * B : (k + 1) * B], wis[k], start=False, stop=(k == KE - 1))
                    nc.scalar.activation(rz[:, c * GW : (c + 1) * GW], ps[:], AF.Sigmoid)
                for c in range(2):
                    cg = c + 4
                    ps_i = psp.tile([B, GW], F32, tag="ps", name="ps_i0")
                    first = True
                    if with_bias:
                        nc.tensor.matmul(ps_i[:], ones_sb[:], bias_sb["in0"][:, c * GW : c * GW + GW], start=True, stop=False)
                        first = False
                    wis = wih0_slices(cg)
                    for k in range(KE):
                        nc.tensor.matmul(ps_i[:], xT[:, k * B : (k + 1) * B], wis[k], start=first, stop=(k == KE - 1))
                        first = False
                    ps_h = psp.tile([B, GW], F32, tag="ps", name="ps_h0")
                    first = True
                    if with_bias:
                        nc.tensor.matmul(ps_h[:], ones_sb[:], bias_sb["hn0"][:, c * GW : c * GW + GW], start=True, stop=False)
                        first = False
                    whs = whh0_slices(cg)
                    for k in range(KH):
                        nc.tensor.matmul(ps_h[:], h0T[:, k * B : (k + 1) * B], whs[k], start=first, stop=(k == KH - 1))
                        first = False
                    tmp = scrs.tile([B, GW], F32, tag="tmp", name="tmp0")
                    nc.vector.tensor_tensor(tmp[:], rz[:, c * GW : (c + 1) * GW], ps_h[:], ALU.mult)
                    nc.vector.tensor_tensor(tmp[:], tmp[:], ps_i[:], ALU.add)
                    n_sb = scrs.tile([B, GW], F32, tag="n_sb", name="n0")
                    nc.scalar.activation(n_sb[:], tmp[:], AF.Tanh)
                    d = scrs.tile([B, GW], F32, tag="d", name="d0")
                    hs = h0[:, c * GW : (c + 1) * GW]
                    nc.vector.tensor_tensor(d[:], hs, n_sb[:], ALU.subtract)
                    nc.vector.tensor_tensor(d[:], rz[:, 1024 + c * GW : 1024 + (c + 1) * GW], d[:], ALU.mult)
                    nc.vector.tensor_tensor(hs, n_sb[:], d[:], ALU.add)
                _transpose_to_T(nc, h0[:], h0T, H)

            for blk in range(n_blocks):
                g_blk = glp.tile([B, INNER * V], F32, tag="g_blk")
                nc.sync.dma_start(out=g_blk[:], in_=g_in[:, blk * INNER * V : (blk + 1) * INNER * V])
                l_blk = glp.tile([B, INNER * V], F32, tag="l_blk")

                with tc.For_i(0, INNER * V, V) as iv:
                    gru_layer0()

                    # layer 1: k-outer streaming from HBM
                    rz1 = scr.tile([B, 2048], F32, tag="rz", name="rz1")
                    psA = [psp.tile([B, GW], F32, tag="ps", name=f"psA{i}") for i in range(4)]
                    if with_bias:
                        for c in range(4):
                            nc.tensor.matmul(psA[c][:], ones_sb[:], bias_sb["rz1"][:, c * GW : (c + 1) * GW], start=True, stop=False)
                    for k in range(KH):
                        whkA = wsp.tile([128, 1024], F32, tag="wstr", name="whkA")
                        nc.sync.dma_start(out=whkA[:], in_=whhT1[:, k * 3 * H : k * 3 * H + 1024])
                        whkB = wsp.tile([128, 1024], F32, tag="wstr", name="whkB")
                        nc.sync.dma_start(out=whkB[:], in_=whhT1[:, k * 3 * H + 1024 : k * 3 * H + 2048])
                        for c in range(4):
                            src = whkA if c < 2 else whkB
                            nc.tensor.matmul(psA[c][:], h1T[:, k * B : (k + 1) * B], src[:, (c % 2) * GW : (c % 2) * GW + GW], start=(k == 0 and not with_bias), stop=False)
                    for k in range(KH):
                        wikA = wsp.tile([128, 1024], F32, tag="wstr", name="wikA")
                        nc.sync.dma_start(out=wikA[:], in_=wihT1[:, k * 3 * H : k * 3 * H + 1024])
                        wikB = wsp.tile([128, 1024], F32, tag="wstr", name="wikB")
                        nc.sync.dma_start(out=wikB[:], in_=wihT1[:, k * 3 * H + 1024 : k * 3 * H + 2048])
                        for c in range(4):
                            src = wikA if c < 2 else wikB
                            nc.tensor.matmul(psA[c][:], h0T[:, k * B : (k + 1) * B], src[:, (c % 2) * GW : (c % 2) * GW + GW], start=False, stop=(k == KH - 1))
                    for c in range(4):
                        nc.scalar.activation(rz1[:, c * GW : (c + 1) * GW], psA[c][:], AF.Sigmoid)

                    psI = [psp.tile([B, GW], F32, tag="ps", name=f"psI{i}") for i in range(2)]
                    psH = [psp.tile([B, GW], F32, tag="ps", name=f"psH{i}") for i in range(2)]
                    if with_bias:
                        for c in range(2):
                            nc.tensor.matmul(psI[c][:], ones_sb[:], bias_sb["in1"][:, c * GW : (c + 1) * GW], start=True, stop=False)
                            nc.tensor.matmul(psH[c][:], ones_sb[:], bias_sb["hn1"][:, c * GW : (c + 1) * GW], start=True, stop=False)
                    for k in range(KH):
                        whkn = wsp.tile([128, 1024], F32, tag="wstr", name="whkn")
                        nc.sync.dma_start(out=whkn[:], in_=whhT1[:, k * 3 * H + 2048 : (k + 1) * 3 * H])
                        for c in range(2):
                            nc.tensor.matmul(psH[c][:], h1T[:, k * B : (k + 1) * B], whkn[:, c * GW : (c + 1) * GW], start=(k == 0 and not with_bias), stop=(k == KH - 1))
                    for k in range(KH):
                        wikn = wsp.tile([128, 1024], F32, tag="wstr", name="wikn")
                        nc.sync.dma_start(out=wikn[:], in_=wihT1[:, k * 3 * H + 2048 : (k + 1) * 3 * H])
                        for c in range(2):
                            nc.tensor.matmul(psI[c][:], h0T[:, k * B : (k + 1) * B], wikn[:, c * GW : (c + 1) * GW], start=(k == 0 and not with_bias), stop=(k == KH - 1))
                    for c in range(2):
                        tmp = scrs.tile([B, GW], F32, tag="tmp", name="tmp1")
                        nc.vector.tensor_tensor(tmp[:], rz1[:, c * GW : (c + 1) * GW], psH[c][:], ALU.mult)
                        nc.vector.tensor_tensor(tmp[:], tmp[:], psI[c][:], ALU.add)
                        n_sb = scrs.tile([B, GW], F32, tag="n_sb", name="n1")
                        nc.scalar.activation(n_sb[:], tmp[:], AF.Tanh)
                        d = scrs.tile([B, GW], F32, tag="d", name="d1")
                        hs = h1[:, c * GW : (c + 1) * GW]
                        nc.vector.tensor_tensor(d[:], hs, n_sb[:], ALU.subtract)
                        nc.vector.tensor_tensor(d[:], rz1[:, 1024 + c * GW : 1024 + (c + 1) * GW], d[:], ALU.mult)
                        nc.vector.tensor_tensor(hs, n_sb[:], d[:], ALU.add)
                    _transpose_to_T(nc, h1[:], h1T, H)

                    # logits
                    ps_l = pssp.tile([B, V], F32, tag="ps_l")
                    first = True
                    if with_bias:
                        nc.tensor.matmul(ps_l[:], ones_sb[:], bias_sb["out"][:], start=True, stop=False)
                        first = False
                    for k in range(KH):
                        nc.tensor.matmul(ps_l[:], h1T[:, k * B : (k + 1) * B], woutT_sb[:, k * V : (k + 1) * V], start=first, stop=(k == KH - 1))
                        first = False
                    nc.scalar.activation(l_blk[:, bass.ds(iv, V)], ps_l[:], AF.Copy)

                    # sample next token as onehot(argmax(logits + g))
                    s_sb = scrs.tile([B, V], F32, tag="s_sb")
                    nc.vector.tensor_tensor(s_sb[:], ps_l[:], g_blk[:, bass.ds(iv, V)], ALU.add)
                    m_sb = scrs.tile([B, 8], F32, tag="m_sb")
                    nc.vector.max(m_sb[:], s_sb[:])
                    nc.vector.tensor_scalar(onehot[:, 0:V], s_sb[:], m_sb[:, 0:1], None, ALU.is_equal)
                    ohT = scrs.tile([32, 32], F32, tag="ohT")
                    nc.vector.transpose(ohT[:], onehot[:])

                    # x = emb[tok] = emb^T @ onehot^T
                    for m in range(KE):
                        ps_x = pssp.tile([128, B], F32, tag="ps_x")
                        nc.tensor.matmul(ps_x[:], embL_sb[:, m * 128 : (m + 1) * 128], ohT[0:V, :], start=True, stop=True)
                        nc.scalar.activation(xT[:, m * B : (m + 1) * B], ps_x[:], AF.Copy)

                nc.sync.dma_start(out=lout[:, blk * INNER * V : (blk + 1) * INNER * V], in_=l_blk[:])

    return nc


# ---------------------------------------------------------------------------
# Host-side prep
# ---------------------------------------------------------------------------

def _chunkT(w, kchunks):
    R_, K = w.shape
    assert K == kchunks * 128
    out = np.empty((128, kchunks * R_), np.float32)
    for k in range(kchunks):
        out[:, k * R_ : (k + 1) * R_] = w[:, k * 128 : (k + 1) * 128].T
    return out


def _gumbel_noise():
    """G [T, 256, V]: the exact noise jax.random.categorical(keys[t], .) uses."""
    import jax
    import jax.numpy as jnp

    cpu = jax.devices("cpu")[0]
    with jax.default_device(cpu):
        keys = jax.random.split(jax.random.key(42), T)
        gfn = jax.jit(lambda k: jax.random.gumbel(k, (BF, V), jnp.float32), backend="cpu")
        return np.stack([np.asarray(gfn(keys[t])) for t in range(T)])


def _host_inputs(inputs, G, core, shared):
    sl = slice(core * B, (core + 1) * B)
    g = np.ascontiguousarray(G[:, sl, :].transpose(1, 0, 2)).reshape(B, T * V)
    m = {"z": np.ascontiguousarray(np.asarray(inputs["z"], np.float32)[sl]), "g": g}
    m.update(shared)
    return m


def _shared_inputs(inputs):
    emb = np.asarray(inputs["emb"], np.float32)
    x0 = emb[SOS]
    x0T = np.empty((128, KE * B), np.float32)
    for k in range(KE):
        x0T[:, k * B : (k + 1) * B] = np.repeat(x0[k * 128 : (k + 1) * 128][:, None], B, axis=1)
    return {
        "wzhT": _chunkT(np.asarray(inputs["w_zh"], np.float32), 8),
        "wihT0": _chunkT(np.asarray(inputs["w_ih0"], np.float32), KE),
        "whhT0": _chunkT(np.asarray(inputs["w_hh0"], np.float32), KH),
        "wihT1": _chunkT(np.asarray(inputs["w_ih1"], np.float32), KH),
        "whhT1": _chunkT(np.asarray(inputs["w_hh1"], np.float32), KH),
        "woutT": _chunkT(np.asarray(inputs["w_out"], np.float32), KH),
        "embL": np.ascontiguousarray(emb),
        "x0T": x0T,
    }


def _bias_inputs(inputs):
    out = {}
    for nm, ki, kh in (("0", "b_ih0", "b_hh0"), ("1", "b_ih1", "b_hh1")):
        bi = np.asarray(inputs[ki], np.float32)
        bh = np.asarray(inputs[kh], np.float32)
        out[f"b_rz{nm}"] = np.ascontiguousarray((bi[: 2 * H] + bh[: 2 * H])[None, :])
        out[f"b_in{nm}"] = np.ascontiguousarray(bi[2 * H :][None, :])
        out[f"b_hn{nm}"] = np.ascontiguousarray(bh[2 * H :][None, :])
    out["b_zh"] = np.ascontiguousarray(np.asarray(inputs["b_zh"], np.float32)[None, :])
    out["b_out"] = np.ascontiguousarray(np.asarray(inputs["b_out"], np.float32)[None, :])
    return out


_CACHE = {}


def kernel(**inputs):
    biases = [inputs[k] for k in ("b_zh", "b_ih0", "b_hh0", "b_ih1", "b_hh1", "b_out")]
    with_bias = any(np.any(np.asarray(b)) for b in biases)

    key = ("nc", with_bias)
    if key not in _CACHE:
        _CACHE[key] = build_decoder(T_=T, INNER=64, with_bias=with_bias)
    nc = _CACHE[key]

    if "G" not in _CACHE:
        _CACHE["G"] = _gumbel_noise()
    G = _CACHE["G"]

    shared = _shared_inputs(inputs)
    if with_bias:
        shared.update(_bias_inputs(inputs))
    in_maps = [_host_inputs(inputs, G, c, shared) for c in range(8)]
    res = run_bass_kernel_spmd(nc, in_maps, list(range(8)))

    logits = np.zeros((BF, T, V), np.float32)
    for c in range(8):
        logits[c * B : (c + 1) * B] = res.results[c]["lout"].reshape(B, T, V)
    tokens = np.argmax(logits.transpose(1, 0, 2) + G, axis=-1).T.astype(np.int32)
    return logits, tokens
